# revision 3
# baseline (speedup 1.0000x reference)
"""Trainium2 Bass kernel for nn_Jitter: out[:, i, :] = x[:, indices[i], :].

Full shapes: x (64, 4096, 256) f32, indices (4096,) int -> out (64, 4096, 256) f32.

Strategy: data-parallel over batch dim across 8 NeuronCores (8 batches per
core); the tiny index vector is replicated to every core. On each core the
time-axis gather uses the SWDGE `dma_gather` ucode instruction (production
embedding-gather path). Work is split into half-batch tiles [128, 16, 256]
(16KB/partition, pool bufs=8) for fine-grained pipelining: per half, 4
gather instructions (512 indices each, 1KB rows) spread round-robin over 4
SWDGE queues pull rows into the tile (index i of the half -> partition
i%128, chunk i//128), and an HWDGE DMA (alternating SP/Activation rings)
stores the tile to the matching interleaved view of that half's output
range. Memory-bound: each core moves 32MB in + 32MB out; measured ~207us/
iter ~= the 64MB shared-DMA-bus roofline (~308 GB/s/core sustained of the
360 GB/s spec). The 4-queue SWDGE split is what buys the bandwidth - a
single queue's descriptor generation serializes at ~298us; the half-batch
tiling shaves the pipeline head/tail versus full-batch tiles (~210-217us).

Indices for dma_gather are int16, wrapped into 16 partitions PER HALF of
2048 (idx i of half h -> partition i%16, col h*128 + i//16) and replicated
to all 128 partitions for the 8 GpSimd cores.
"""

import contextlib

import numpy as np

import concourse.bass as bass
import concourse.tile as tile
from concourse import bacc, mybir
from concourse.bass_utils import run_bass_kernel_spmd
from concourse.library_config import mlp as _mlp_lib

N_CORES = 8
B, T, C = 64, 4096, 256
B_LOC = B // N_CORES  # 8 batches per core
P = 128               # SBUF partitions
J = T // P            # 32 gathered rows per partition (full batch)
JW = T // 16          # idx tile cols (16-partition wrap)

_CACHE = {}

N_SWDGE_QUEUES = 4
H = 2                 # half-batch tiles
JH = J // H           # 16 rows per partition per half
TH = T // H           # 2048 indices per half
G_PER_H = 4           # gather instructions per half
IDX_PER_G = TH // G_PER_H      # 512 indices per gather instruction
JW_H = TH // 16                # idx cols per half
JW_PER_G = JW_H // G_PER_H
J_PER_G = JH // G_PER_H


def _build(repeat: int = 1, bufs: int = 8):
    """Build + compile the per-core SPMD program.

    repeat: run the whole gather body `repeat` times inside a hardware
            For_i loop (for delta timing in test harnesses); the result
            is unchanged.
    """
    nc = bacc.Bacc("TRN2", target_bir_lowering=False, debug=False,
                   num_devices=N_CORES, num_swdge_queues=N_SWDGE_QUEUES)
    x_ext = nc.dram_tensor("x", [B_LOC, T, C], mybir.dt.float32,
                           kind="ExternalInput").ap()
    idx_ext = nc.dram_tensor("idx", [P, JW], mybir.dt.int16,
                             kind="ExternalInput").ap()
    out_ext = nc.dram_tensor("out", [B_LOC, T, C], mybir.dt.float32,
                             kind="ExternalOutput").ap()

    with tile.TileContext(nc) as tc:
        with tc.tile_pool(name="idxp", bufs=1) as idx_pool, \
             tc.tile_pool(name="data", bufs=bufs) as data_pool:
            nc.gpsimd.load_library(_mlp_lib)
            idx_t = idx_pool.tile([P, JW], mybir.dt.int16)
            nc.sync.dma_start(out=idx_t[:], in_=idx_ext[:])
            loop = tc.For_i(0, repeat) if repeat > 1 else contextlib.nullcontext()
            with loop:
                for b in range(B_LOC):
                    for h in range(H):
                        dt = data_pool.tile([P, JH, C], mybir.dt.float32)
                        base = h * JW_H
                        for g in range(G_PER_H):
                            # half-local index i in [g*512, (g+1)*512) lands
                            # at [i % 128, i // 128] of the tile
                            nc.gpsimd.dma_gather(
                                dt[:, g * J_PER_G:(g + 1) * J_PER_G, :],
                                x_ext[b],
                                idx_t[:, base + g * JW_PER_G:
                                         base + (g + 1) * JW_PER_G],
                                num_idxs=IDX_PER_G, num_idxs_reg=IDX_PER_G,
                                elem_size=C,
                                queue_num=(b * H + h + g) % N_SWDGE_QUEUES,
                            )
                        # tile slot (p, j) holds x[idx[h*2048 + j*128 + p]]
                        out_view = out_ext[b, h * TH:(h + 1) * TH].rearrange(
                            "(j p) c -> p j c", p=P)
                        eng_s = nc.sync if (b * H + h) % 2 == 0 else nc.scalar
                        eng_s.dma_start(out=out_view, in_=dt[:])
    nc.compile()
    return nc


def _prep_idx(indices: np.ndarray) -> np.ndarray:
    """Wrap each half's 2048 indices into 16 partitions, concat halves
    along cols, replicate to 128 partitions."""
    idx16 = np.asarray(indices).astype(np.int16)        # values < 4096 fit
    parts = []
    for h in range(H):
        seg = idx16[h * TH:(h + 1) * TH]
        parts.append(np.ascontiguousarray(seg.reshape(JW_H, 16).T))  # [16, JW_H]
    full = np.concatenate(parts, axis=1)                # [16, JW]
    return np.ascontiguousarray(np.tile(full, (P // 16, 1)))  # [128, JW]


def kernel(x: np.ndarray, indices: np.ndarray) -> np.ndarray:
    key = "main"
    if key not in _CACHE:
        _CACHE[key] = _build()
    nc = _CACHE[key]

    idx_arr = _prep_idx(np.asarray(indices))
    x = np.asarray(x)
    in_maps = [
        {"x": np.ascontiguousarray(x[i * B_LOC:(i + 1) * B_LOC]),
         "idx": idx_arr}
        for i in range(N_CORES)
    ]
    res = run_bass_kernel_spmd(nc, in_maps, list(range(N_CORES)))
    return np.concatenate([res.results[i]["out"] for i in range(N_CORES)],
                          axis=0)


# revision 4
# speedup vs baseline: 1.0138x; 1.0138x over previous
"""Trainium2 Bass kernel for nn_Jitter: out[:, i, :] = x[:, indices[i], :].

Full shapes: x (64, 4096, 256) f32, indices (4096,) int -> out (64, 4096, 256) f32.

Strategy: data-parallel over batch dim across 8 NeuronCores (8 batches per
core); the tiny index vector is replicated to every core. On each core the
time-axis gather uses the SWDGE `dma_gather` ucode instruction (production
embedding-gather path). Work is split into half-batch tiles [128, 16, 256]
(16KB/partition, pool bufs=8) for fine-grained pipelining: per half, 4
gather instructions (512 indices each, 1KB rows) spread round-robin over 4
SWDGE queues pull rows into the tile (index i of the half -> partition
i%128, chunk i//128), and an HWDGE DMA (alternating SP/Activation rings)
stores the tile to the matching interleaved view of that half's output
range. Memory-bound: each core moves 32MB in + 32MB out; measured ~207us/
iter ~= the 64MB shared-DMA-bus roofline (~308 GB/s/core sustained of the
360 GB/s spec). The 4-queue SWDGE split is what buys the bandwidth - a
single queue's descriptor generation serializes at ~298us; the half-batch
tiling shaves the pipeline head/tail versus full-batch tiles (~210-217us).

Indices for dma_gather are int16, wrapped into 16 partitions PER HALF of
2048 (idx i of half h -> partition i%16, col h*128 + i//16) and replicated
to all 128 partitions for the 8 GpSimd cores.
"""

import contextlib

import numpy as np

import concourse.bass as bass
import concourse.tile as tile
from concourse import bacc, mybir
from concourse.bass_utils import run_bass_kernel_spmd
from concourse.library_config import mlp as _mlp_lib

N_CORES = 8
B, T, C = 64, 4096, 256
B_LOC = B // N_CORES  # 8 batches per core
P = 128               # SBUF partitions
J = T // P            # 32 gathered rows per partition (full batch)
JW = T // 16          # idx tile cols (16-partition wrap)

_CACHE = {}

N_SWDGE_QUEUES = 4
H = 2                 # half-batch tiles
JH = J // H           # 16 rows per partition per half
TH = T // H           # 2048 indices per half
G_PER_H = 4           # gather instructions per half
IDX_PER_G = TH // G_PER_H      # 512 indices per gather instruction
JW_H = TH // 16                # idx cols per half
JW_PER_G = JW_H // G_PER_H
J_PER_G = JH // G_PER_H


def _build(repeat: int = 1, bufs: int = 8):
    """Build + compile the per-core SPMD program.

    repeat: run the whole gather body `repeat` times inside a hardware
            For_i loop (for delta timing in test harnesses); the result
            is unchanged.
    """
    nc = bacc.Bacc("TRN2", target_bir_lowering=False, debug=False,
                   num_devices=N_CORES, num_swdge_queues=N_SWDGE_QUEUES)
    x_ext = nc.dram_tensor("x", [B_LOC, T, C], mybir.dt.float32,
                           kind="ExternalInput").ap()
    idx_ext = nc.dram_tensor("idx", [P, JW], mybir.dt.int16,
                             kind="ExternalInput").ap()
    out_ext = nc.dram_tensor("out", [B_LOC, T, C], mybir.dt.float32,
                             kind="ExternalOutput").ap()

    with tile.TileContext(nc) as tc:
        with tc.tile_pool(name="idxp", bufs=1) as idx_pool, \
             tc.tile_pool(name="data", bufs=bufs) as data_pool:
            nc.gpsimd.load_library(_mlp_lib)
            idx_t = idx_pool.tile([P, JW], mybir.dt.int16)
            nc.sync.dma_start(out=idx_t[:], in_=idx_ext[:])
            loop = tc.For_i(0, repeat) if repeat > 1 else contextlib.nullcontext()
            with loop:
                for b in range(B_LOC):
                    for h in range(H):
                        dt = data_pool.tile([P, JH, C], mybir.dt.float32)
                        base = h * JW_H
                        for g in range(G_PER_H):
                            # half-local index i in [g*512, (g+1)*512) lands
                            # at [i % 128, i // 128] of the tile
                            nc.gpsimd.dma_gather(
                                dt[:, g * J_PER_G:(g + 1) * J_PER_G, :],
                                x_ext[b],
                                idx_t[:, base + g * JW_PER_G:
                                         base + (g + 1) * JW_PER_G],
                                num_idxs=IDX_PER_G, num_idxs_reg=IDX_PER_G,
                                elem_size=C,
                                queue_num=(b * H + h + g) % N_SWDGE_QUEUES,
                            )
                        # tile slot (p, j) holds x[idx[h*2048 + j*128 + p]]
                        out_view = out_ext[b, h * TH:(h + 1) * TH].rearrange(
                            "(j p) c -> p j c", p=P)
                        eng_s = nc.sync if (b * H + h) % 2 == 0 else nc.scalar
                        eng_s.dma_start(out=out_view, in_=dt[:],
                                        single_packet=True)
    nc.compile()
    return nc


def _prep_idx(indices: np.ndarray) -> np.ndarray:
    """Wrap each half's 2048 indices into 16 partitions, concat halves
    along cols, replicate to 128 partitions."""
    idx16 = np.asarray(indices).astype(np.int16)        # values < 4096 fit
    parts = []
    for h in range(H):
        seg = idx16[h * TH:(h + 1) * TH]
        parts.append(np.ascontiguousarray(seg.reshape(JW_H, 16).T))  # [16, JW_H]
    full = np.concatenate(parts, axis=1)                # [16, JW]
    return np.ascontiguousarray(np.tile(full, (P // 16, 1)))  # [128, JW]


def kernel(x: np.ndarray, indices: np.ndarray) -> np.ndarray:
    key = "main"
    if key not in _CACHE:
        _CACHE[key] = _build()
    nc = _CACHE[key]

    idx_arr = _prep_idx(np.asarray(indices))
    x = np.asarray(x)
    in_maps = [
        {"x": np.ascontiguousarray(x[i * B_LOC:(i + 1) * B_LOC]),
         "idx": idx_arr}
        for i in range(N_CORES)
    ]
    res = run_bass_kernel_spmd(nc, in_maps, list(range(N_CORES)))
    return np.concatenate([res.results[i]["out"] for i in range(N_CORES)],
                          axis=0)
